# revision 18
# baseline (speedup 1.0000x reference)
"""Trainium2 Bass kernel for nn_AggregationEncoder (gnn_message_passing).

Reference computation:
    adj[g, m] = 1 where an edge (g, m) exists (set semantics, duplicate
                edges collapse to 1)
    norm[m]   = max(sum_g adj[g, m], 1)
    out[b, m, d] = sum_g adj[g, m] / norm[m] * x[b, g, d]

Structural facts hardcoded from the problem spec:
  - x: [B=2, G=40962, D=512] float32
  - edge_index: [E=122880, 2] int64, BOTH columns drawn from [0, 2562),
    so the adjacency has nonzero rows only for g < 2562 and the einsum
    only needs x[:, :2562, :] (rows >= 2562 multiply zero adjacency).
  - M = 2562 mesh nodes.

Sharding (8 cores): 2 batches x 4 mesh-column chunks of W=672 columns
(mesh axis padded to 2688 = 4*672). Host work is sharding only: slice x
per batch, partition the (dedup'd) edge list by receiver chunk and pack
it as per-sender-row receiver lists (a CSR-like sharded layout).

Device-side (per core):
  1. GPSIMD local_scatter builds the 0/1 adjacency chunk directly in
     SBUF, three [128, 672] k-planes per instruction (zero-fill plus
     1.0 writes at receiver indices; -1 slots are ignored).
  2. x loads fp32 via HWDGE in 3-k-tile chunks; ScalarE casts to bf16
     (keeps the DVE<->GpSimd shared SBUF port free for the scatters).
  3. VectorE accumulates s2[p, m] = sum_kt A[kt*128+p, m] behind the
     scatter pipeline (counts <= 21, exact in bf16).
  4. TensorE: psum[mt] += A_kt^T @ x_kt, k-tiles in pairs per PSUM-bank
     visit; degree matmuls (s2^T @ ones) run mid-stream once s2 lands
     so the reciprocals are ready before the last accumulation stops.
  5. VectorE/ScalarE: out = psum * (1/max(deg,1)), alternating engines.
  6. DMA out [672, 512] fp32; host reassembles [2, 2562, 512].
"""

import numpy as np

B = 2
G = 40962
D = 512
M = 2562          # mesh nodes
GP = 2688         # padded sender rows = 21*128 (all senders < 2562)
KT = GP // 128    # 21 k-tiles
NQ = 4            # mesh-column chunks
W = 672           # mesh columns per chunk (4*672 = 2688 >= 2562)
MT_SIZES = [128, 128, 128, 128, 128, 32]  # 672 = 5*128 + 32
PLANES = 3        # k-planes per local_scatter (num_elems = 3*672 = 2016)
NSC = KT // PLANES  # 7 scatter instructions
R3 = 76           # receiver slots per (partition, 3-plane group), pad -1
N_CORES = 8

_NC_CACHE = None


def _build_bass():
    import concourse.bacc as bacc
    import concourse.mybir as mybir
    import concourse.tile as tile
    from concourse import library_config

    dt = mybir.dt
    nc = bacc.Bacc("TRN2", target_bir_lowering=False, debug=False,
                   num_devices=N_CORES)

    xfeat = nc.dram_tensor("xfeat", [GP, D], dt.float32, kind="ExternalInput")
    ridx = nc.dram_tensor("ridx", [128, NSC * R3], dt.int16,
                          kind="ExternalInput")
    out = nc.dram_tensor("out", [W, D], dt.float32, kind="ExternalOutput")

    NMT = len(MT_SIZES)

    with tile.TileContext(nc) as tc:
        with (
            tc.tile_pool(name="sbuf", bufs=1) as sb,
            tc.tile_pool(name="outb", bufs=2) as outb,
            tc.tile_pool(name="psum", bufs=1, space="PSUM") as ps,
        ):
            # Kick the GPSIMD library code-load immediately — no data deps,
            # ~4.5us; otherwise it serializes behind the first scatter's
            # input waits.
            nc.gpsimd.load_library(library_config.local_scatter)

            ridx_sb = sb.tile([128, NSC * R3], dt.int16)
            nc.sync.dma_start(ridx_sb[:], ridx[:])
            ones_data = sb.tile([128, R3], dt.bfloat16)
            nc.vector.memset(ones_data[:], 1.0)
            one_col = sb.tile([128, 1], dt.bfloat16)
            nc.vector.memset(one_col[:], 1.0)

            # ---- adjacency build in SBUF via GPSIMD local scatter ----
            a_sb = sb.tile([128, KT, W], dt.bfloat16)
            for sc in range(NSC):
                nc.gpsimd.local_scatter(
                    out_ap=a_sb[:, sc * PLANES:(sc + 1) * PLANES, :],
                    data_ap=ones_data[:],
                    idxs_ap=ridx_sb[:, sc * R3:(sc + 1) * R3],
                    channels=128,
                    num_elems=PLANES * W,
                    num_idxs=R3,
                )

            # ---- x load (fp32 HWDGE, 3-k-tile chunks) + ScalarE cast ----
            x32_sb = sb.tile([128, KT, D], dt.float32)
            x_sb = sb.tile([128, KT, D], dt.bfloat16)
            for sc in range(NSC):
                k0 = sc * PLANES
                nc.sync.dma_start(
                    out=x32_sb[:, k0:k0 + PLANES, :],
                    in_=xfeat[k0 * 128:(k0 + PLANES) * 128, :].rearrange(
                        "(k p) d -> p k d", p=128),
                )
                nc.scalar.activation(
                    x_sb[:, k0:k0 + PLANES, :], x32_sb[:, k0:k0 + PLANES, :],
                    mybir.ActivationFunctionType.Copy)

            # ---- main matmuls (kt triples per PSUM-bank visit, matching
            # scatter granularity) ----
            psums = [ps.tile([128, D], dt.float32, tag=f"ps{mt}",
                             name=f"psum{mt}")
                     for mt in range(NMT)]
            s2 = sb.tile([128, W], dt.bfloat16)

            # Warm-up matmuls: keep TensorE busy through the head (~7..14us)
            # so HAM is at full clock when the real stream starts. Dedicated
            # PSUM slot (shares the pdeg0 tag, which is only used much
            # later); N=512 so each dummy holds the PE ~216ns.
            warm_src = sb.tile([128, D], dt.bfloat16)
            nc.vector.memset(warm_src[:], 1.0)
            warm = ps.tile([128, D], dt.float32, tag="pdeg0", name="warm")
            for _ in range(13):
                nc.tensor.matmul(
                    warm[0:32, :],
                    lhsT=ones_data[:, 0:32],
                    rhs=warm_src[:],
                    start=True,
                    stop=True,
                )

            def mm_group(kts):
                col = 0
                for mt, msz in enumerate(MT_SIZES):
                    for kt in kts:
                        nc.tensor.matmul(
                            psums[mt][:msz, :],
                            lhsT=a_sb[:, kt, col:col + msz],
                            rhs=x_sb[:, kt, :],
                            start=(kt == 0),
                            stop=(kt == KT - 1),
                        )
                    col += msz

            for t in range(6):  # kts 0..17 in triples
                mm_group([3 * t, 3 * t + 1, 3 * t + 2])

            # s2 accumulation (all 21 planes) must be fully issued before
            # the degree matmuls read it; each add just waits on its scatter.
            for kt in range(KT):
                if kt == 0:
                    nc.vector.tensor_copy(s2[:], a_sb[:, 0, :])
                else:
                    nc.vector.tensor_add(s2[:], s2[:], a_sb[:, kt, :])

            # ---- degree + reciprocal, mid-stream (s2 ready by now) ----
            degs = []
            col = 0
            for mt, msz in enumerate(MT_SIZES):
                pdeg = ps.tile([128, 1], dt.float32, tag=f"pdeg{mt % 2}",
                               name=f"pdeg{mt}")
                nc.tensor.matmul(
                    pdeg[:msz, :],
                    lhsT=s2[:, col:col + msz],
                    rhs=one_col[:],
                    start=True,
                    stop=True,
                )
                deg = outb.tile([128, 1], dt.float32, tag=f"deg{mt % 2}",
                                name=f"deg{mt}")
                nc.vector.tensor_scalar_max(deg[:msz], pdeg[:msz, 0:1], 1.0)
                recip = outb.tile([128, 1], dt.float32, tag=f"recip{mt}",
                                  name=f"recip{mt}")
                nc.vector.reciprocal(recip[:msz], deg[:msz])
                degs.append(recip)
                col += msz

            mm_group([18, 19, 20])

            # ---- normalize + store (alternate DVE / ScalarE) ----
            col = 0
            for mt, msz in enumerate(MT_SIZES):
                recip = degs[mt]
                o_sb = outb.tile([128, D], dt.float32, tag=f"osb{mt % 2}",
                                 name=f"osb{mt}")
                if mt % 2 == 0:
                    nc.vector.tensor_scalar_mul(
                        o_sb[:msz], psums[mt][:msz, 0:D], recip[:msz, 0:1])
                else:
                    nc.scalar.activation(
                        o_sb[:msz], psums[mt][:msz, 0:D],
                        mybir.ActivationFunctionType.Copy,
                        scale=recip[:msz, 0:1])
                nc.sync.dma_start(out[col:col + msz, :], o_sb[:msz])
                col += msz

    nc.finalize()
    return nc


def _get_nc():
    global _NC_CACHE
    if _NC_CACHE is None:
        _NC_CACHE = _build_bass()
    return _NC_CACHE


def _shard_edges(edge_index):
    """Partition edges by receiver chunk and pack per-(sender-row,
    3-plane-group) receiver lists: per-chunk int16 [128, NSC*R3], pad -1.
    Index value = local_m + (kt % PLANES) * W."""
    e = np.asarray(edge_index)
    g = e[:, 0].astype(np.int64)
    m = e[:, 1].astype(np.int64)
    # dedup (g, m) pairs — set semantics of the reference adjacency
    key = np.unique(g * M + m)
    g = key // M
    m = key % M
    kt = g >> 7
    p = g & 127
    grp = kt // PLANES
    sub = kt % PLANES
    slot = p * NSC + grp          # [0, 128*NSC)
    shards = []
    for q in range(NQ):
        lo, hi = q * W, (q + 1) * W
        mask = (m >= lo) & (m < hi)
        sq = slot[mask]
        vq = ((m[mask] - lo) + sub[mask] * W).astype(np.int16)
        order = np.argsort(sq, kind="stable")
        sq = sq[order]
        vq = vq[order]
        counts = np.bincount(sq, minlength=128 * NSC)
        if counts.max() > R3:
            raise ValueError(
                f"chunk {q}: slot has {counts.max()} receivers > R3={R3}")
        starts = np.zeros(128 * NSC, dtype=np.int64)
        starts[1:] = np.cumsum(counts)[:-1]
        rank = np.arange(sq.shape[0]) - starts[sq]
        arr = np.full((128 * NSC, R3), -1, dtype=np.int16)
        arr[sq, rank] = vq
        shards.append(np.ascontiguousarray(arr.reshape(128, NSC * R3)))
    return shards


def kernel(grid_node_features, edge_index):
    from concourse.bass_utils import run_bass_kernel_spmd

    nc = _get_nc()
    x = np.asarray(grid_node_features)
    shards = _shard_edges(edge_index)
    xb = [np.ascontiguousarray(x[b, :GP, :], dtype=np.float32)
          for b in range(B)]

    in_maps = []
    for c in range(N_CORES):
        b, q = divmod(c, NQ)
        in_maps.append({"xfeat": xb[b], "ridx": shards[q]})

    res = run_bass_kernel_spmd(nc, in_maps, core_ids=list(range(N_CORES)))

    out = np.empty((B, M, D), dtype=np.float32)
    for c in range(N_CORES):
        b, q = divmod(c, NQ)
        lo = q * W
        hi = min(lo + W, M)
        out[b, lo:hi, :] = res.results[c]["out"][:hi - lo, :]
    return out


# revision 19
# speedup vs baseline: 1.0322x; 1.0322x over previous
"""Trainium2 Bass kernel for nn_AggregationEncoder (gnn_message_passing).

Reference computation:
    adj[g, m] = 1 where an edge (g, m) exists (set semantics, duplicate
                edges collapse to 1)
    norm[m]   = max(sum_g adj[g, m], 1)
    out[b, m, d] = sum_g adj[g, m] / norm[m] * x[b, g, d]

Structural facts hardcoded from the problem spec:
  - x: [B=2, G=40962, D=512] float32
  - edge_index: [E=122880, 2] int64, BOTH columns drawn from [0, 2562),
    so the adjacency has nonzero rows only for g < 2562 and the einsum
    only needs x[:, :2562, :] (rows >= 2562 multiply zero adjacency).
  - M = 2562 mesh nodes.

Sharding (8 cores): 2 batches x 4 mesh-column chunks of W=672 columns
(mesh axis padded to 2688 = 4*672). Host work is sharding only: slice x
per batch, partition the (dedup'd) edge list by receiver chunk and pack
it as per-sender-row receiver lists (a CSR-like sharded layout).

Device-side (per core):
  1. GPSIMD local_scatter builds the 0/1 adjacency chunk directly in
     SBUF, three [128, 672] k-planes per instruction (zero-fill plus
     1.0 writes at receiver indices; -1 slots are ignored).
  2. x loads fp32 via HWDGE in 3-k-tile chunks; ScalarE casts to bf16
     (keeps the DVE<->GpSimd shared SBUF port free for the scatters).
  3. VectorE accumulates s2[p, m] = sum_kt A[kt*128+p, m] behind the
     scatter pipeline (counts <= 21, exact in bf16).
  4. TensorE: psum[mt] += A_kt^T @ x_kt, k-tiles in pairs per PSUM-bank
     visit; degree matmuls (s2^T @ ones) run mid-stream once s2 lands
     so the reciprocals are ready before the last accumulation stops.
  5. VectorE/ScalarE: out = psum * (1/max(deg,1)), alternating engines.
  6. DMA out [672, 512] fp32; host reassembles [2, 2562, 512].
"""

import numpy as np

B = 2
G = 40962
D = 512
M = 2562          # mesh nodes
GP = 2688         # padded sender rows = 21*128 (all senders < 2562)
KT = GP // 128    # 21 k-tiles
NQ = 4            # mesh-column chunks
W = 672           # mesh columns per chunk (4*672 = 2688 >= 2562)
MT_SIZES = [128, 128, 128, 128, 128, 32]  # 672 = 5*128 + 32
PLANES = 3        # k-planes per local_scatter (num_elems = 3*672 = 2016)
NSC = KT // PLANES  # 7 scatter instructions
R3 = 76           # receiver slots per (partition, 3-plane group), pad -1
N_CORES = 8

_NC_CACHE = None


def _build_bass():
    import concourse.bacc as bacc
    import concourse.mybir as mybir
    import concourse.tile as tile
    from concourse import library_config

    dt = mybir.dt
    nc = bacc.Bacc("TRN2", target_bir_lowering=False, debug=False,
                   num_devices=N_CORES)

    xfeat = nc.dram_tensor("xfeat", [GP, D], dt.float32, kind="ExternalInput")
    ridx = nc.dram_tensor("ridx", [128, NSC * R3], dt.int16,
                          kind="ExternalInput")
    out = nc.dram_tensor("out", [W, D], dt.float32, kind="ExternalOutput")

    NMT = len(MT_SIZES)

    with tile.TileContext(nc) as tc:
        with (
            tc.tile_pool(name="sbuf", bufs=1) as sb,
            tc.tile_pool(name="outb", bufs=2) as outb,
            tc.tile_pool(name="psum", bufs=1, space="PSUM") as ps,
        ):
            # Kick the GPSIMD library code-load immediately — no data deps,
            # ~4.5us; otherwise it serializes behind the first scatter's
            # input waits.
            nc.gpsimd.load_library(library_config.local_scatter)

            ridx_sb = sb.tile([128, NSC * R3], dt.int16)
            nc.sync.dma_start(ridx_sb[:], ridx[:])
            ones_data = sb.tile([128, R3], dt.bfloat16)
            nc.vector.memset(ones_data[:], 1.0)
            one_col = sb.tile([128, 1], dt.bfloat16)
            nc.vector.memset(one_col[:], 1.0)

            # ---- adjacency build in SBUF via GPSIMD local scatter ----
            a_sb = sb.tile([128, KT, W], dt.bfloat16)
            for sc in range(NSC):
                nc.gpsimd.local_scatter(
                    out_ap=a_sb[:, sc * PLANES:(sc + 1) * PLANES, :],
                    data_ap=ones_data[:],
                    idxs_ap=ridx_sb[:, sc * R3:(sc + 1) * R3],
                    channels=128,
                    num_elems=PLANES * W,
                    num_idxs=R3,
                )

            # ---- x load (fp32 HWDGE, 3-k-tile chunks) + ScalarE cast ----
            x32_sb = sb.tile([128, KT, D], dt.float32)
            x_sb = sb.tile([128, KT, D], dt.bfloat16)
            for sc in range(NSC):
                k0 = sc * PLANES
                nc.sync.dma_start(
                    out=x32_sb[:, k0:k0 + PLANES, :],
                    in_=xfeat[k0 * 128:(k0 + PLANES) * 128, :].rearrange(
                        "(k p) d -> p k d", p=128),
                )
                nc.scalar.activation(
                    x_sb[:, k0:k0 + PLANES, :], x32_sb[:, k0:k0 + PLANES, :],
                    mybir.ActivationFunctionType.Copy)

            # ---- main matmuls (kt triples per PSUM-bank visit, matching
            # scatter granularity) ----
            psums = [ps.tile([128, D], dt.float32, tag=f"ps{mt}",
                             name=f"psum{mt}")
                     for mt in range(NMT)]
            s2 = sb.tile([128, W], dt.bfloat16)

            # Warm-up matmuls: keep TensorE busy through the head (~7..14us)
            # so HAM is at full clock when the real stream starts. Dedicated
            # PSUM slot (shares the pdeg0 tag, which is only used much
            # later); N=512 so each dummy holds the PE ~216ns.
            warm_src = sb.tile([128, D], dt.bfloat16)
            nc.vector.memset(warm_src[:], 1.0)
            warm = ps.tile([128, D], dt.float32, tag="pdeg0", name="warm")
            for _ in range(16):
                nc.tensor.matmul(
                    warm[0:32, :],
                    lhsT=ones_data[:, 0:32],
                    rhs=warm_src[:],
                    start=True,
                    stop=True,
                )

            def mm_group(kts):
                col = 0
                for mt, msz in enumerate(MT_SIZES):
                    for kt in kts:
                        nc.tensor.matmul(
                            psums[mt][:msz, :],
                            lhsT=a_sb[:, kt, col:col + msz],
                            rhs=x_sb[:, kt, :],
                            start=(kt == 0),
                            stop=(kt == KT - 1),
                        )
                    col += msz

            for t in range(6):  # kts 0..17 in triples
                mm_group([3 * t, 3 * t + 1, 3 * t + 2])

            # s2 accumulation (all 21 planes) must be fully issued before
            # the degree matmuls read it; each add just waits on its scatter.
            for kt in range(KT):
                if kt == 0:
                    nc.vector.tensor_copy(s2[:], a_sb[:, 0, :])
                else:
                    nc.vector.tensor_add(s2[:], s2[:], a_sb[:, kt, :])

            # ---- degree + reciprocal, mid-stream (s2 ready by now) ----
            degs = []
            col = 0
            for mt, msz in enumerate(MT_SIZES):
                pdeg = ps.tile([128, 1], dt.float32, tag=f"pdeg{mt % 2}",
                               name=f"pdeg{mt}")
                nc.tensor.matmul(
                    pdeg[:msz, :],
                    lhsT=s2[:, col:col + msz],
                    rhs=one_col[:],
                    start=True,
                    stop=True,
                )
                deg = outb.tile([128, 1], dt.float32, tag=f"deg{mt % 2}",
                                name=f"deg{mt}")
                nc.vector.tensor_scalar_max(deg[:msz], pdeg[:msz, 0:1], 1.0)
                recip = outb.tile([128, 1], dt.float32, tag=f"recip{mt}",
                                  name=f"recip{mt}")
                nc.vector.reciprocal(recip[:msz], deg[:msz])
                degs.append(recip)
                col += msz

            mm_group([18, 19, 20])

            # ---- normalize + store (alternate DVE / ScalarE) ----
            col = 0
            for mt, msz in enumerate(MT_SIZES):
                recip = degs[mt]
                o_sb = outb.tile([128, D], dt.float32, tag=f"osb{mt % 2}",
                                 name=f"osb{mt}")
                if mt % 2 == 0:
                    nc.vector.tensor_scalar_mul(
                        o_sb[:msz], psums[mt][:msz, 0:D], recip[:msz, 0:1])
                else:
                    nc.scalar.activation(
                        o_sb[:msz], psums[mt][:msz, 0:D],
                        mybir.ActivationFunctionType.Copy,
                        scale=recip[:msz, 0:1])
                nc.sync.dma_start(out[col:col + msz, :], o_sb[:msz])
                col += msz

    nc.finalize()
    return nc


def _get_nc():
    global _NC_CACHE
    if _NC_CACHE is None:
        _NC_CACHE = _build_bass()
    return _NC_CACHE


def _shard_edges(edge_index):
    """Partition edges by receiver chunk and pack per-(sender-row,
    3-plane-group) receiver lists: per-chunk int16 [128, NSC*R3], pad -1.
    Index value = local_m + (kt % PLANES) * W."""
    e = np.asarray(edge_index)
    g = e[:, 0].astype(np.int64)
    m = e[:, 1].astype(np.int64)
    # dedup (g, m) pairs — set semantics of the reference adjacency
    key = np.unique(g * M + m)
    g = key // M
    m = key % M
    kt = g >> 7
    p = g & 127
    grp = kt // PLANES
    sub = kt % PLANES
    slot = p * NSC + grp          # [0, 128*NSC)
    shards = []
    for q in range(NQ):
        lo, hi = q * W, (q + 1) * W
        mask = (m >= lo) & (m < hi)
        sq = slot[mask]
        vq = ((m[mask] - lo) + sub[mask] * W).astype(np.int16)
        order = np.argsort(sq, kind="stable")
        sq = sq[order]
        vq = vq[order]
        counts = np.bincount(sq, minlength=128 * NSC)
        if counts.max() > R3:
            raise ValueError(
                f"chunk {q}: slot has {counts.max()} receivers > R3={R3}")
        starts = np.zeros(128 * NSC, dtype=np.int64)
        starts[1:] = np.cumsum(counts)[:-1]
        rank = np.arange(sq.shape[0]) - starts[sq]
        arr = np.full((128 * NSC, R3), -1, dtype=np.int16)
        arr[sq, rank] = vq
        shards.append(np.ascontiguousarray(arr.reshape(128, NSC * R3)))
    return shards


def kernel(grid_node_features, edge_index):
    from concourse.bass_utils import run_bass_kernel_spmd

    nc = _get_nc()
    x = np.asarray(grid_node_features)
    shards = _shard_edges(edge_index)
    xb = [np.ascontiguousarray(x[b, :GP, :], dtype=np.float32)
          for b in range(B)]

    in_maps = []
    for c in range(N_CORES):
        b, q = divmod(c, NQ)
        in_maps.append({"xfeat": xb[b], "ridx": shards[q]})

    res = run_bass_kernel_spmd(nc, in_maps, core_ids=list(range(N_CORES)))

    out = np.empty((B, M, D), dtype=np.float32)
    for c in range(N_CORES):
        b, q = divmod(c, NQ)
        lo = q * W
        hi = min(lo + W, M)
        out[b, lo:hi, :] = res.results[c]["out"][:hi - lo, :]
    return out
